# revision 4
# baseline (speedup 1.0000x reference)
"""MoE router kernel (GPT-OSS TopK router) for 8 Trainium2 NeuronCores.

Computation (per reference):
    logits = hidden_states @ weight.T + bias        # [T=16384, E=128]
    top_v, top_i = top_k(logits, 4)                 # [T, 4]
    top_v = softmax(top_v, axis=-1)
    return (top_v, top_i, logits)

Sharding: token dim split 8 ways (2048 tokens/core); weight+bias replicated.
The x shard and weight are fed to the device pre-transposed ([H, T] / [H, E])
so the contraction dim lands on SBUF partitions with fully-contiguous DMA.

Device pipeline per core, per 512-token group:
  DMA xT group -> 24-chunk matmul accumulation into PSUM [128 tok, 128 exp]
  (bias folded in as a K=1 matmul against a ones-row) -> ACT copy to SBUF ->
  DMA logits out; DVE max8/max_index for top-4 values+indices; ACT exp with
  per-partition bias (-max) and accumulated sum; DVE reciprocal + scale.
"""

import sys

for _p in ("/opt/trn_rl_repo",):
    if _p not in sys.path:
        sys.path.insert(0, _p)

import numpy as np

import concourse.bass as bass
import concourse.mybir as mybir
from concourse import bacc
from concourse.bass_utils import run_bass_kernel_spmd
from concourse.tile import TileContext

# Problem shape (hardcoded per contract)
T = 16384
H = 2880
E = 128
K = 4
N_CORES = 8
TC = T // N_CORES  # tokens per core = 2048

# Tiling
TG = 512  # tokens per DMA group
N_GROUPS = TC // TG  # 4
SUBT = TG // 128  # matmul subtiles per group = 4
HC = H // 128  # full 128-row contraction chunks = 22 (rem 64)
HREM = H - HC * 128  # 64
N_TILES = TC // 128  # 16 token tiles per core

FP32 = mybir.dt.float32
I32 = mybir.dt.int32
U32 = mybir.dt.uint32


def _build_program():
    nc = bacc.Bacc("TRN2", target_bir_lowering=False, debug=False)

    xT = nc.declare_dram_parameter("xT", [H, TC], FP32, isOutput=False).ap()
    wT = nc.declare_dram_parameter("wT", [H, E], FP32, isOutput=False).ap()
    bias1 = nc.declare_dram_parameter("bias1", [1, E], FP32, isOutput=False).ap()
    out_logits = nc.declare_dram_parameter(
        "out_logits", [TC, E], FP32, isOutput=True
    ).ap()
    out_vals = nc.declare_dram_parameter("out_vals", [TC, K], FP32, isOutput=True).ap()
    out_idx = nc.declare_dram_parameter("out_idx", [TC, K], I32, isOutput=True).ap()

    with TileContext(nc) as tc:
        with (
            tc.tile_pool(name="wpool", bufs=1) as wpool,
            tc.tile_pool(name="xpool", bufs=2) as xpool,
            tc.tile_pool(name="xrem", bufs=2) as xrempool,
            tc.tile_pool(name="psum", bufs=4, space="PSUM") as psum,
            tc.tile_pool(name="lpool", bufs=3) as lpool,
            tc.tile_pool(name="small", bufs=4) as small,
            tc.tile_pool(name="stage", bufs=1) as stage,
        ):
            # One-time loads: transposed weight, bias, ones row
            wt_main = wpool.tile([128, HC, E], FP32)
            nc.sync.dma_start(
                wt_main, wT[: HC * 128, :].rearrange("(c p) e -> p c e", p=128)
            )
            wt_rem = wpool.tile([HREM, E], FP32)
            nc.sync.dma_start(wt_rem, wT[HC * 128 :, :])
            bias_sb = wpool.tile([1, E], FP32)
            nc.sync.dma_start(bias_sb, bias1)
            ones = wpool.tile([1, 128], FP32)
            nc.vector.memset(ones, 1.0)

            vals_stage = stage.tile([128, N_TILES, K], FP32)
            idx_stage = stage.tile([128, N_TILES, K], I32)

            for g in range(N_GROUPS):
                t0 = g * TG
                xg = xpool.tile([128, HC, TG], FP32)
                nc.sync.dma_start(
                    xg,
                    xT[: HC * 128, t0 : t0 + TG].rearrange("(c p) t -> p c t", p=128),
                )
                xg_rem = xrempool.tile([HREM, TG], FP32)
                nc.sync.dma_start(xg_rem, xT[HC * 128 :, t0 : t0 + TG])

                for s in range(SUBT):
                    j = g * SUBT + s  # token tile index (0..15)
                    tok = j * 128
                    ts = slice(s * 128, (s + 1) * 128)

                    ps = psum.tile([128, E], FP32)
                    for c in range(HC):
                        nc.tensor.matmul(
                            ps,
                            lhsT=xg[:, c, ts],
                            rhs=wt_main[:, c, :],
                            start=(c == 0),
                            stop=False,
                        )
                    nc.tensor.matmul(
                        ps, lhsT=xg_rem[:, ts], rhs=wt_rem, start=False, stop=False
                    )
                    # bias: ones[t].T @ bias[e] accumulated on top (K=1 matmul)
                    nc.tensor.matmul(
                        ps, lhsT=ones, rhs=bias_sb, start=False, stop=True
                    )

                    logits_sb = lpool.tile([128, E], FP32)
                    nc.scalar.copy(logits_sb, ps)
                    nc.sync.dma_start(out_logits[tok : tok + 128, :], logits_sb)

                    top8v = small.tile([128, 8], FP32)
                    nc.vector.max(top8v, logits_sb)
                    top8i = small.tile([128, 8], U32)
                    nc.vector.max_index(top8i, top8v, logits_sb)

                    negmax = small.tile([128, 1], FP32)
                    nc.vector.tensor_scalar_mul(negmax, top8v[:, 0:1], -1.0)
                    expv = small.tile([128, K], FP32)
                    sum4 = small.tile([128, 1], FP32)
                    nc.scalar.activation(
                        expv,
                        top8v[:, 0:K],
                        mybir.ActivationFunctionType.Exp,
                        bias=negmax,
                        scale=1.0,
                        accum_out=sum4,
                    )
                    rsum = small.tile([128, 1], FP32)
                    nc.vector.reciprocal(rsum, sum4)
                    nc.vector.tensor_scalar_mul(
                        vals_stage[:, j, :], expv, rsum
                    )
                    nc.vector.tensor_copy(idx_stage[:, j, :], top8i[:, 0:K])

            nc.sync.dma_start(
                out_vals.rearrange("(j p) k -> p j k", p=128), vals_stage
            )
            nc.sync.dma_start(
                out_idx.rearrange("(j p) k -> p j k", p=128), idx_stage
            )

    nc.finalize()
    return nc


_PROGRAM_CACHE = {}


def _get_program():
    if "nc" not in _PROGRAM_CACHE:
        _PROGRAM_CACHE["nc"] = _build_program()
    return _PROGRAM_CACHE["nc"]


def kernel(hidden_states, weight, bias, _trace=False, _trace_kwargs=None):
    x = np.ascontiguousarray(np.asarray(hidden_states, dtype=np.float32))
    w = np.ascontiguousarray(np.asarray(weight, dtype=np.float32))
    b = np.ascontiguousarray(np.asarray(bias, dtype=np.float32))
    assert x.shape == (T, H) and w.shape == (E, H) and b.shape == (E,)

    wT = np.ascontiguousarray(w.T)
    bias1 = b.reshape(1, E)
    in_maps = []
    for i in range(N_CORES):
        shard = x[i * TC : (i + 1) * TC, :]
        in_maps.append(
            {
                "xT": np.ascontiguousarray(shard.T),
                "wT": wT,
                "bias1": bias1,
            }
        )

    nc = _get_program()
    kw = {}
    if _trace:
        kw = dict(trace=True, **(_trace_kwargs or {}))
    br = run_bass_kernel_spmd(nc, in_maps, list(range(N_CORES)), **kw)
    results = br.results

    vals = np.concatenate([results[i]["out_vals"] for i in range(N_CORES)], axis=0)
    idx = np.concatenate([results[i]["out_idx"] for i in range(N_CORES)], axis=0)
    logits = np.concatenate(
        [results[i]["out_logits"] for i in range(N_CORES)], axis=0
    )
    if _trace:
        return (vals, idx.astype(np.int32), logits), br
    return (vals, idx.astype(np.int32), logits)


# revision 5
# speedup vs baseline: 1.1581x; 1.1581x over previous
"""MoE router kernel (GPT-OSS TopK router) for 8 Trainium2 NeuronCores.

Computation (per reference):
    logits = hidden_states @ weight.T + bias        # [T=16384, E=128]
    top_v, top_i = top_k(logits, 4)                 # [T, 4]
    top_v = softmax(top_v, axis=-1)
    return (top_v, top_i, logits)

Sharding: token dim split 8 ways (2048 tokens/core); weight+bias replicated.
The x shard and weight are fed to the device pre-transposed ([H, T] / [H, E])
so the contraction dim lands on SBUF partitions with fully-contiguous DMA.
The bias is folded in by appending a ones-row to xT and a bias-row to wT
(H -> 2881 rows), so logits = xT_pad.T @ wT_pad exactly.

Device pipeline per core, per 512-token group:
  DMA xT group; 23-chunk matmul accumulation with the WEIGHT as the
  stationary operand (LDWEIGHTS reused across the 512-token moving side)
  into PSUM logitsT [128 exp, 512 tok]; ACT copy to SBUF; PE-transpose
  each 128-token tile back to [tok, exp]; ACT copy; DMA logits out; DVE
  max8/find_index8 for top-4 values+indices; ACT exp (bias=-max,
  accumulated sum); DVE reciprocal + scale. Scalar outputs are staged in
  SBUF and written once at the end.
"""

import sys

for _p in ("/opt/trn_rl_repo",):
    if _p not in sys.path:
        sys.path.insert(0, _p)

import numpy as np

import concourse.bass as bass
import concourse.mybir as mybir
from concourse import bacc
from concourse.bass_utils import run_bass_kernel_spmd
from concourse.masks import make_identity
from concourse.tile import TileContext

# Problem shape (hardcoded per contract)
T = 16384
H = 2880
E = 128
K = 4
N_CORES = 8
TC = T // N_CORES  # tokens per core = 2048

HP = H + 1  # padded contraction dim: ones/bias row folded in
HC = H // 128  # full 128-row contraction chunks = 22
HREM = HP - HC * 128  # 65 (64 x-rows + ones row)
HSPLIT = 11  # chunks per x DMA (two half-loads per group)

TG = 512  # tokens per group (PSUM bank free-dim)
N_GROUPS = TC // TG  # 4
SUBT = TG // 128  # 128-token subtiles per group = 4
N_TILES = TC // 128  # 16 token tiles per core

FP32 = mybir.dt.float32
I32 = mybir.dt.int32
U32 = mybir.dt.uint32


def _build_program():
    nc = bacc.Bacc("TRN2", target_bir_lowering=False, debug=False)

    xT = nc.declare_dram_parameter("xT", [HP, TC], FP32, isOutput=False).ap()
    wT = nc.declare_dram_parameter("wT", [HP, E], FP32, isOutput=False).ap()
    out_logits = nc.declare_dram_parameter(
        "out_logits", [TC, E], FP32, isOutput=True
    ).ap()
    out_vals = nc.declare_dram_parameter("out_vals", [TC, K], FP32, isOutput=True).ap()
    out_idx = nc.declare_dram_parameter("out_idx", [TC, K], I32, isOutput=True).ap()

    with TileContext(nc) as tc:
        with (
            tc.tile_pool(name="wpool", bufs=1) as wpool,
            tc.tile_pool(name="xpool", bufs=2) as xpool,
            tc.tile_pool(name="xrem", bufs=2) as xrempool,
            tc.tile_pool(name="psg", bufs=2, space="PSUM") as psg,
            tc.tile_pool(name="pst", bufs=4, space="PSUM") as pst,
            tc.tile_pool(name="ltpool", bufs=2) as ltpool,
            tc.tile_pool(name="lpool", bufs=3) as lpool,
            tc.tile_pool(name="small", bufs=4) as small,
            tc.tile_pool(name="stage", bufs=1) as stage,
        ):
            # One-time loads: transposed weight(+bias row), identity
            wt_main = wpool.tile([128, HC, E], FP32)
            nc.sync.dma_start(
                wt_main, wT[: HC * 128, :].rearrange("(c p) e -> p c e", p=128)
            )
            wt_rem = wpool.tile([HREM, E], FP32)
            nc.sync.dma_start(wt_rem, wT[HC * 128 :, :])
            identity = wpool.tile([128, 128], FP32)
            make_identity(nc, identity)

            vals_stage = stage.tile([128, N_TILES, K], FP32)
            idx_stage = stage.tile([128, N_TILES, K], I32)

            for g in range(N_GROUPS):
                t0 = g * TG
                xa = xpool.tile([128, HSPLIT, TG], FP32, tag="xa")
                nc.sync.dma_start(
                    xa,
                    xT[: HSPLIT * 128, t0 : t0 + TG].rearrange(
                        "(c p) t -> p c t", p=128
                    ),
                )
                xb = xpool.tile([128, HC - HSPLIT, TG], FP32, tag="xb")
                nc.sync.dma_start(
                    xb,
                    xT[HSPLIT * 128 : HC * 128, t0 : t0 + TG].rearrange(
                        "(c p) t -> p c t", p=128
                    ),
                )
                xr = xrempool.tile([HREM, TG], FP32)
                nc.sync.dma_start(xr, xT[HC * 128 :, t0 : t0 + TG])

                # logitsT[e, t] accumulated over 23 chunks; weight stationary
                ps = psg.tile([128, TG], FP32)
                for c in range(HC):
                    src = xa[:, c, :] if c < HSPLIT else xb[:, c - HSPLIT, :]
                    nc.tensor.matmul(
                        ps,
                        lhsT=wt_main[:, c, :],
                        rhs=src,
                        start=(c == 0),
                        stop=False,
                    )
                nc.tensor.matmul(ps, lhsT=wt_rem, rhs=xr, start=False, stop=True)

                ltT = ltpool.tile([128, TG], FP32)
                nc.scalar.copy(ltT, ps)

                for s in range(SUBT):
                    j = g * SUBT + s  # token tile index (0..15)
                    tok = j * 128

                    ps_t = pst.tile([128, 128], FP32)
                    nc.tensor.transpose(
                        ps_t, ltT[:, s * 128 : (s + 1) * 128], identity
                    )
                    logits_sb = lpool.tile([128, E], FP32)
                    nc.scalar.copy(logits_sb, ps_t)
                    nc.sync.dma_start(out_logits[tok : tok + 128, :], logits_sb)

                    top8v = small.tile([128, 8], FP32)
                    nc.vector.max(top8v, logits_sb)
                    top8i = small.tile([128, 8], U32)
                    nc.vector.max_index(top8i, top8v, logits_sb)

                    negmax = small.tile([128, 1], FP32)
                    nc.vector.tensor_scalar_mul(negmax, top8v[:, 0:1], -1.0)
                    expv = small.tile([128, K], FP32)
                    sum4 = small.tile([128, 1], FP32)
                    nc.scalar.activation(
                        expv,
                        top8v[:, 0:K],
                        mybir.ActivationFunctionType.Exp,
                        bias=negmax,
                        scale=1.0,
                        accum_out=sum4,
                    )
                    rsum = small.tile([128, 1], FP32)
                    nc.vector.reciprocal(rsum, sum4)
                    nc.vector.tensor_scalar_mul(vals_stage[:, j, :], expv, rsum)
                    nc.vector.tensor_copy(idx_stage[:, j, :], top8i[:, 0:K])

            nc.sync.dma_start(
                out_vals.rearrange("(j p) k -> p j k", p=128), vals_stage
            )
            nc.sync.dma_start(
                out_idx.rearrange("(j p) k -> p j k", p=128), idx_stage
            )

    nc.finalize()
    return nc


_PROGRAM_CACHE = {}


def _get_program():
    if "nc" not in _PROGRAM_CACHE:
        _PROGRAM_CACHE["nc"] = _build_program()
    return _PROGRAM_CACHE["nc"]


def kernel(hidden_states, weight, bias, _trace=False, _trace_kwargs=None):
    x = np.asarray(hidden_states, dtype=np.float32)
    w = np.asarray(weight, dtype=np.float32)
    b = np.asarray(bias, dtype=np.float32)
    assert x.shape == (T, H) and w.shape == (E, H) and b.shape == (E,)

    wTp = np.empty((HP, E), dtype=np.float32)
    wTp[:H] = w.T
    wTp[H] = b

    in_maps = []
    for i in range(N_CORES):
        xTp = np.empty((HP, TC), dtype=np.float32)
        xTp[:H] = x[i * TC : (i + 1) * TC, :].T
        xTp[H] = 1.0
        in_maps.append({"xT": xTp, "wT": wTp})

    nc = _get_program()
    kw = {}
    if _trace:
        kw = dict(trace=True, **(_trace_kwargs or {}))
    br = run_bass_kernel_spmd(nc, in_maps, list(range(N_CORES)), **kw)
    results = br.results

    vals = np.concatenate([results[i]["out_vals"] for i in range(N_CORES)], axis=0)
    idx = np.concatenate([results[i]["out_idx"] for i in range(N_CORES)], axis=0)
    logits = np.concatenate(
        [results[i]["out_logits"] for i in range(N_CORES)], axis=0
    )
    if _trace:
        return (vals, idx.astype(np.int32), logits), br
    return (vals, idx.astype(np.int32), logits)


# revision 7
# speedup vs baseline: 1.2057x; 1.0411x over previous
"""MoE router kernel (GPT-OSS TopK router) for 8 Trainium2 NeuronCores.

Computation (per reference):
    logits = hidden_states @ weight.T + bias        # [T=16384, E=128]
    top_v, top_i = top_k(logits, 4)                 # [T, 4]
    top_v = softmax(top_v, axis=-1)
    return (top_v, top_i, logits)

Sharding: token dim split 8 ways (2048 tokens/core); weight+bias replicated.
The x shard and weight are fed to the device pre-transposed and pre-tiled
into the exact per-group chunk order the kernel consumes, so every DMA
reads DRAM sequentially with 2KB descriptors. The bias is folded in by
appending a ones-row to xT and a bias-row to wT (H -> 2881 rows), so
logits = xT_pad.T @ wT_pad exactly (fp32 — float32r flips top-k indices).

Device pipeline per core, per 512-token group:
  4 sub-DMAs load the group's x chunks; 23-chunk fp32 matmul accumulation
  with the WEIGHT stationary into PSUM logitsT [128 exp, 512 tok]; ACT
  copy to SBUF; PE-transpose each 128-token tile back to [tok, exp]; ACT
  copy; DMA logits out; DVE max8/find_index8 for top-4 values+indices;
  ACT exp (bias=-max, accumulated sum); DVE reciprocal + scale. Scalar
  outputs are staged in SBUF and written once at the end.
"""

import sys

for _p in ("/opt/trn_rl_repo",):
    if _p not in sys.path:
        sys.path.insert(0, _p)

import numpy as np

import concourse.bass as bass
import concourse.mybir as mybir
from concourse import bacc
from concourse.bass_utils import run_bass_kernel_spmd
from concourse.masks import make_identity
from concourse.tile import TileContext

# Problem shape (hardcoded per contract)
T = 16384
H = 2880
E = 128
K = 4
N_CORES = 8
TC = T // N_CORES  # tokens per core = 2048

HP = H + 1  # padded contraction dim: ones/bias row folded in
HC = H // 128  # full 128-row contraction chunks = 22
HREM = HP - HC * 128  # 65 (64 x-rows + ones row)
# chunk ranges per x sub-DMA (earlier matmul start + finer DMA pipelining)
XSPLITS = [(0, 6), (6, 12), (12, 17), (17, 22)]

TG = 512  # tokens per group (PSUM bank free-dim)
N_GROUPS = TC // TG  # 4
SUBT = TG // 128  # 128-token subtiles per group = 4
N_TILES = TC // 128  # 16 token tiles per core

FP32 = mybir.dt.float32
I32 = mybir.dt.int32
U32 = mybir.dt.uint32


def _build_program():
    nc = bacc.Bacc("TRN2", target_bir_lowering=False, debug=False)

    # x main chunks, pre-tiled on host: [group, chunk, partition, token]
    xm = nc.declare_dram_parameter(
        "xm", [N_GROUPS, HC, 128, TG], FP32, isOutput=False
    ).ap()
    # x remainder rows (incl. ones row): [group, row, token]
    xr_d = nc.declare_dram_parameter(
        "xr", [N_GROUPS, HREM, TG], FP32, isOutput=False
    ).ap()
    wT = nc.declare_dram_parameter("wT", [HP, E], FP32, isOutput=False).ap()
    out_logits = nc.declare_dram_parameter(
        "out_logits", [TC, E], FP32, isOutput=True
    ).ap()
    out_vals = nc.declare_dram_parameter("out_vals", [TC, K], FP32, isOutput=True).ap()
    out_idx = nc.declare_dram_parameter("out_idx", [TC, K], I32, isOutput=True).ap()

    with TileContext(nc) as tc:
        with (
            tc.tile_pool(name="wpool", bufs=1) as wpool,
            tc.tile_pool(name="xpool", bufs=2) as xpool,
            tc.tile_pool(name="xrem", bufs=2) as xrempool,
            tc.tile_pool(name="psg", bufs=2, space="PSUM") as psg,
            tc.tile_pool(name="pst", bufs=4, space="PSUM") as pst,
            tc.tile_pool(name="ltpool", bufs=2) as ltpool,
            tc.tile_pool(name="lpool", bufs=3) as lpool,
            tc.tile_pool(name="small", bufs=4) as small,
            tc.tile_pool(name="stage", bufs=1) as stage,
        ):
            # One-time loads: transposed weight(+bias row), identity
            wt_main = wpool.tile([128, HC, E], FP32)
            nc.sync.dma_start(
                wt_main, wT[: HC * 128, :].rearrange("(c p) e -> p c e", p=128)
            )
            wt_rem = wpool.tile([HREM, E], FP32)
            nc.sync.dma_start(wt_rem, wT[HC * 128 :, :])
            identity = wpool.tile([128, 128], FP32)
            make_identity(nc, identity)

            vals_stage = stage.tile([128, N_TILES, K], FP32)
            idx_stage = stage.tile([128, N_TILES, K], I32)

            for g in range(N_GROUPS):
                xs = []
                for si, (c0, c1) in enumerate(XSPLITS):
                    xt = xpool.tile([128, c1 - c0, TG], FP32, tag=f"xs{si}")
                    nc.sync.dma_start(
                        xt, xm[g, c0:c1].rearrange("c p t -> p c t")
                    )
                    xs.append(xt)
                xr = xrempool.tile([HREM, TG], FP32)
                nc.sync.dma_start(xr, xr_d[g])

                # logitsT[e, t] accumulated over 23 chunks; weight stationary
                ps = psg.tile([128, TG], FP32)
                for si, (c0, c1) in enumerate(XSPLITS):
                    for c in range(c0, c1):
                        nc.tensor.matmul(
                            ps,
                            lhsT=wt_main[:, c, :],
                            rhs=xs[si][:, c - c0, :],
                            start=(c == 0),
                            stop=False,
                        )
                nc.tensor.matmul(ps, lhsT=wt_rem, rhs=xr, start=False, stop=True)

                ltT = ltpool.tile([128, TG], FP32)
                nc.scalar.copy(ltT, ps)

                for s in range(SUBT):
                    j = g * SUBT + s  # token tile index (0..15)
                    tok = j * 128

                    ps_t = pst.tile([128, 128], FP32)
                    nc.tensor.transpose(
                        ps_t, ltT[:, s * 128 : (s + 1) * 128], identity
                    )
                    logits_sb = lpool.tile([128, E], FP32)
                    nc.scalar.copy(logits_sb, ps_t)
                    nc.sync.dma_start(out_logits[tok : tok + 128, :], logits_sb)

                    top8v = small.tile([128, 8], FP32)
                    nc.vector.max(top8v, logits_sb)
                    top8i = small.tile([128, 8], U32)
                    nc.vector.max_index(top8i, top8v, logits_sb)

                    negmax = small.tile([128, 1], FP32)
                    nc.vector.tensor_scalar_mul(negmax, top8v[:, 0:1], -1.0)
                    expv = small.tile([128, K], FP32)
                    sum4 = small.tile([128, 1], FP32)
                    nc.scalar.activation(
                        expv,
                        top8v[:, 0:K],
                        mybir.ActivationFunctionType.Exp,
                        bias=negmax,
                        scale=1.0,
                        accum_out=sum4,
                    )
                    rsum = small.tile([128, 1], FP32)
                    nc.vector.reciprocal(rsum, sum4)
                    nc.vector.tensor_scalar_mul(vals_stage[:, j, :], expv, rsum)
                    nc.vector.tensor_copy(idx_stage[:, j, :], top8i[:, 0:K])

            nc.sync.dma_start(
                out_vals.rearrange("(j p) k -> p j k", p=128), vals_stage
            )
            nc.sync.dma_start(
                out_idx.rearrange("(j p) k -> p j k", p=128), idx_stage
            )

    nc.finalize()
    return nc


_PROGRAM_CACHE = {}


def _get_program():
    if "nc" not in _PROGRAM_CACHE:
        _PROGRAM_CACHE["nc"] = _build_program()
    return _PROGRAM_CACHE["nc"]


def kernel(hidden_states, weight, bias, _trace=False, _trace_kwargs=None):
    x = np.asarray(hidden_states, dtype=np.float32)
    w = np.asarray(weight, dtype=np.float32)
    b = np.asarray(bias, dtype=np.float32)
    assert x.shape == (T, H) and w.shape == (E, H) and b.shape == (E,)

    wTp = np.empty((HP, E), dtype=np.float32)
    wTp[:H] = w.T
    wTp[H] = b

    in_maps = []
    for i in range(N_CORES):
        shard = x[i * TC : (i + 1) * TC, :]  # [TC, H]
        # [HC*128, TC] -> [HC, 128, N_GROUPS, TG] -> [N_GROUPS, HC, 128, TG]
        xmain = np.ascontiguousarray(
            shard.T[: HC * 128]
            .reshape(HC, 128, N_GROUPS, TG)
            .transpose(2, 0, 1, 3)
        )
        xrem = np.empty((N_GROUPS, HREM, TG), dtype=np.float32)
        xrem[:, : H - HC * 128, :] = (
            shard.T[HC * 128 :].reshape(H - HC * 128, N_GROUPS, TG).transpose(1, 0, 2)
        )
        xrem[:, H - HC * 128 :, :] = 1.0  # ones row (bias)
        in_maps.append({"xm": xmain, "xr": xrem, "wT": wTp})

    nc = _get_program()
    kw = {}
    if _trace:
        kw = dict(trace=True, **(_trace_kwargs or {}))
    br = run_bass_kernel_spmd(nc, in_maps, list(range(N_CORES)), **kw)
    results = br.results

    vals = np.concatenate([results[i]["out_vals"] for i in range(N_CORES)], axis=0)
    idx = np.concatenate([results[i]["out_idx"] for i in range(N_CORES)], axis=0)
    logits = np.concatenate(
        [results[i]["out_logits"] for i in range(N_CORES)], axis=0
    )
    if _trace:
        return (vals, idx.astype(np.int32), logits), br
    return (vals, idx.astype(np.int32), logits)
